# revision 14
# baseline (speedup 1.0000x reference)
"""Trainium2 Bass kernel for nn_CharacterGNN (gnn_message_passing).

The reference network is entirely linear (two PyG GraphConv layers over the
fixed 2-node graph 0<->1, mean over nodes, then a Linear head).  Folding the
linear algebra gives, per sample i with node indices (a_i, b_i):

    out[i] = M @ (emb[a_i] + emb[b_i]) + c

where
    M = 0.5 * fc_W @ (W2_rel + W2_root) @ (W1_rel + W1_root)   # [2, 128]
    c = fc_W @ ((W2_rel + W2_root) @ b1 + b2) + fc_b           # [2]

Device work = a 131072-row random gather from the 1M x 128 table plus a tiny
[*,128] @ [128,2] contraction -- memory bound.

Distribution (table-sharded, per the row-shard variant of the hint):
  - The table is row-sharded: core m owns rows [m*125000, (m+1)*125000),
    stored fp16 and zero-padded to 131072 rows (4 bins of 32768).
  - Every gathered row is routed (on host) to the core owning it; each core
    gathers its ~16384 rows with FOUR dma_gather instructions (one per
    32768-row bin -- dma_gather indices are int16) in TRANSPOSE mode, which
    lands rows channel-major: G_T[chan, col] in SBUF.
  - PE matmuls reduce each 128-column chunk: out[col, o] = sum_c G_T[c, col]
    * M_T[c, o] -- no DVE work at all.
  - Host scatters per-row results y'[row] back to sample order and does the
    final pair add out[s] = y'[2s] + y'[2s+1] + c (131072 adds on 1 MB --
    negligible host work; the memory-heavy gather and the 67 MFLOP reduction
    stay on device).

Why not indirect_dma_start: on real HW it consumes ONE index per partition
per instruction (the multi-index-per-partition semantics exist only in the
simulator), so bulk gathers would need 128 instructions/core and die on
SWDGE fixed overhead.  dma_gather is the purpose-built bulk gather.
"""

import numpy as np

NUM_NODES = 1_000_000
EMB = 128
B = 65536
NCORES = 8
P = 128
SHARD = NUM_NODES // NCORES       # 125000 rows owned per core
BINROWS = 32768                   # int16 index range per dma_gather
NBINS = 4                         # 4*32768 = 131072 >= 125000 (shard padded)
NROWS_PAD = NBINS * BINROWS       # 131072
BINPAD = 4608                     # per-bin index capacity (36*128, mult of 16)
NCOLS = NBINS * BINPAD            # 18432 gather columns per core
NCHUNK = BINPAD // P              # 36 matmul chunks per bin

_CACHE: dict = {}


def _build_nc(binpad: int = BINPAD, reps: int = 1):
    """Build the single-core Bass module (same program runs SPMD on 8 cores).

    reps > 1 wraps the gather+matmul+store pipeline in a tc.For_i loop --
    benchmarking only (wall-clock slope over reps isolates per-iteration HW
    time from NEFF-launch/tunnel overhead)."""
    import contextlib
    import concourse.bass as bass
    import concourse.bacc as bacc
    import concourse.mybir as mybir
    import concourse.tile as tile

    nchunk = binpad // P
    # Bacc (not plain Bass): its compile() runs generate_event_semaphores,
    # which splits multi-wait sync_infos -- TRN2 allows at most 1 wait per
    # instruction and walrus codegen hard-errors otherwise.
    nc = bacc.Bacc("TRN2", target_bir_lowering=False, debug=False)
    embs = nc.dram_tensor("embs", [NROWS_PAD, EMB], mybir.dt.float16,
                          kind="ExternalInput")
    idxs_d = nc.dram_tensor("idxs", [P, NBINS * binpad // 16], mybir.dt.int16,
                            kind="ExternalInput")
    mcat_d = nc.dram_tensor("mcat", [P, 2], mybir.dt.float16, kind="ExternalInput")
    y_d = nc.dram_tensor("y", [P, NBINS * nchunk * 2], mybir.dt.float32,
                         kind="ExternalOutput")

    with tile.TileContext(nc) as tc:
        with tc.tile_pool(name="const", bufs=1) as cpool, \
             tc.tile_pool(name="gat", bufs=2) as gpool, \
             tc.tile_pool(name="ps", bufs=2, space="PSUM") as ppool:

            idxs_sb = cpool.tile([P, NBINS * binpad // 16], mybir.dt.int16)
            mcat_sb = cpool.tile([P, 2], mybir.dt.float16)
            y_sb = cpool.tile([P, NBINS * nchunk * 2], mybir.dt.float32)

            nc.sync.dma_start(out=idxs_sb[:], in_=idxs_d.ap())
            nc.sync.dma_start(out=mcat_sb[:], in_=mcat_d.ap())

            loop_cm = tc.For_i(0, reps, 1) if reps > 1 else contextlib.nullcontext()
            with loop_cm:
                for b in range(NBINS):
                    gt = gpool.tile([P, 1, binpad], mybir.dt.float16, tag="g")
                    nc.gpsimd.dma_gather(
                        out_ap=gt[:],
                        in_ap=embs.ap()[b * BINROWS:(b + 1) * BINROWS, :],
                        idxs_ap=idxs_sb[:, b * (binpad // 16):(b + 1) * (binpad // 16)],
                        num_idxs=binpad,
                        num_idxs_reg=binpad,
                        elem_size=EMB,
                        transpose=True,
                        # single_packet packs one gather's descriptors into one
                        # DMA packet; packets cap at 64 descriptors (16 idxs
                        # each), so >896 idxs requires multi-packet mode --
                        # single_packet=True hard-crashes the NEFF at 4608.
                        single_packet=False,
                    )
                    pt = ppool.tile([P, nchunk * 2], mybir.dt.float32, tag="p")
                    for j in range(nchunk):
                        nc.tensor.matmul(
                            out=pt[:, 2 * j:2 * j + 2],
                            lhsT=gt[:, 0, j * P:(j + 1) * P],
                            rhs=mcat_sb[:],
                            start=True, stop=True)
                    nc.vector.tensor_copy(
                        out=y_sb[:, b * nchunk * 2:(b + 1) * nchunk * 2],
                        in_=pt[:])
                nc.sync.dma_start(out=y_d.ap(), in_=y_sb[:])

    nc.compile()
    return nc


def _fold_weights(W1_rel, b1, W1_root, W2_rel, b2, W2_root, fc_W, fc_b):
    M2 = (W2_rel + W2_root).astype(np.float64)
    M1 = (W1_rel + W1_root).astype(np.float64)
    M = 0.5 * fc_W.astype(np.float64) @ M2 @ M1            # [2, 128]
    c = (fc_W.astype(np.float64) @ (M2 @ b1.astype(np.float64)
                                    + b2.astype(np.float64))
         + fc_b.astype(np.float64))                        # [2]
    return M.astype(np.float32), c.astype(np.float32)


def _route(x, binpad: int = BINPAD):
    """Host-side routing: per core, bin-compacted int16 index arrays plus the
    column -> flat-row-slot map for reassembly.  Returns (in-parts, col_pos
    list, spill list of (slot, row) that exceeded a bin's capacity)."""
    xf = np.ascontiguousarray(np.asarray(x).astype(np.int64).reshape(-1))  # [131072]
    core = xf // SHARD
    idxs_list, colpos_list, spills = [], [], []
    for m in range(NCORES):
        sel = np.nonzero(core == m)[0]
        loc = xf[sel] - m * SHARD                       # [nm] in [0, 125000)
        b = loc // BINROWS                              # bin 0..3
        order = np.argsort(b, kind="stable")
        loc_o, pos_o, b_o = loc[order], sel[order], b[order]
        idx16 = np.zeros((NBINS, binpad), np.int16)
        colpos = np.full((NBINS, binpad), -1, np.int64)
        for bb in range(NBINS):
            seg = loc_o[b_o == bb] - bb * BINROWS
            pp = pos_o[b_o == bb]
            n = len(seg)
            if n > binpad:                              # ~impossible; exact fallback
                for s_loc, s_pos in zip(seg[binpad:], pp[binpad:]):
                    spills.append((int(s_pos), int(m * SHARD + bb * BINROWS + s_loc)))
                seg, pp, n = seg[:binpad], pp[:binpad], binpad
            idx16[bb, :n] = seg.astype(np.int16)
            colpos[bb, :n] = pp
        # dma_gather consumes indices wrapped over 16 partitions: flat index i
        # lives at [i % 16, i // 16]; replicate to all 8 groups of 16.
        wrapped = np.concatenate(
            [idx16[bb].reshape(binpad // 16, 16).T for bb in range(NBINS)], axis=1)
        idxs_list.append(np.tile(wrapped, (8, 1)).astype(np.int16))  # [128, NBINS*binpad/16]
        colpos_list.append(colpos)
    return idxs_list, colpos_list, spills


def _per_core_inputs(x, emb16, M, binpad: int = BINPAD):
    mcat = np.ascontiguousarray(M.T.astype(np.float16))    # [128, 2]
    idxs_list, colpos_list, spills = _route(x, binpad)
    in_maps = []
    for m in range(NCORES):
        embs_m = np.zeros((NROWS_PAD, EMB), np.float16)
        embs_m[:SHARD] = emb16[m * SHARD:(m + 1) * SHARD]
        in_maps.append({
            "embs": embs_m,
            "idxs": idxs_list[m],
            "mcat": mcat,
        })
    return in_maps, colpos_list, spills


def _assemble(y_list, colpos_list, spills, emb, M, c, binpad: int = BINPAD,
              n_rows: int = B * 2):
    nchunk = binpad // P
    yv = np.zeros((n_rows, 2), np.float32)
    for m in range(NCORES):
        # y[p, (b*nchunk + j)*2 + o] = y' of column b*binpad + j*128 + p
        y4 = y_list[m].reshape(P, NBINS, nchunk, 2)
        yflat = y4.transpose(1, 2, 0, 3).reshape(NBINS * binpad, 2)
        cp = colpos_list[m].reshape(-1)
        valid = cp >= 0
        yv[cp[valid]] = yflat[valid]
    for slot, row in spills:
        yv[slot] = (M.astype(np.float64) @ emb[row].astype(np.float64)).astype(np.float32)
    out = yv[0::2] + yv[1::2] + c.astype(np.float32)
    return out


def _get_compiled(binpad: int = BINPAD, reps: int = 1):
    key = ("nc", binpad, reps)
    if key not in _CACHE:
        _CACHE[key] = _build_nc(binpad, reps)
    return _CACHE[key]


def run(x, emb, M, c, trace=False, **spmd_kwargs):
    """Run the SPMD kernel on 8 cores; returns (out [65536,2] f32, results)."""
    from concourse.bass_utils import run_bass_kernel_spmd
    nc = _get_compiled()
    emb16 = emb.astype(np.float16)
    in_maps, colpos_list, spills = _per_core_inputs(x, emb16, M)
    res = run_bass_kernel_spmd(nc, in_maps, core_ids=list(range(NCORES)),
                               trace=trace, **spmd_kwargs)
    y_list = [res.results[m]["y"] for m in range(NCORES)]
    out = _assemble(y_list, colpos_list, spills, emb, M, c)
    return out, res


def kernel(x, emb, W1_rel, b1, W1_root, W2_rel, b2, W2_root, fc_W, fc_b):
    x = np.asarray(x)
    emb = np.asarray(emb, dtype=np.float32)
    M, c = _fold_weights(np.asarray(W1_rel), np.asarray(b1), np.asarray(W1_root),
                         np.asarray(W2_rel), np.asarray(b2), np.asarray(W2_root),
                         np.asarray(fc_W), np.asarray(fc_b))
    out, _ = run(x, emb, M, c)
    return out


# revision 15
# speedup vs baseline: 1.2463x; 1.2463x over previous
"""Trainium2 Bass kernel for nn_CharacterGNN (gnn_message_passing).

The reference network is entirely linear (two PyG GraphConv layers over the
fixed 2-node graph 0<->1, mean over nodes, then a Linear head).  Folding the
linear algebra gives, per sample i with node indices (a_i, b_i):

    out[i] = M @ (emb[a_i] + emb[b_i]) + c

where
    M = 0.5 * fc_W @ (W2_rel + W2_root) @ (W1_rel + W1_root)   # [2, 128]
    c = fc_W @ ((W2_rel + W2_root) @ b1 + b2) + fc_b           # [2]

Device work = a 131072-row random gather from the 1M x 128 table plus a tiny
[*,128] @ [128,2] contraction -- memory bound.

Distribution (table-sharded, per the row-shard variant of the hint):
  - The table is row-sharded: core m owns rows [m*125000, (m+1)*125000),
    stored fp16 and zero-padded to 131072 rows (4 bins of 32768).
  - Every gathered row is routed (on host) to the core owning it; each core
    gathers its ~16384 rows with FOUR dma_gather instructions (one per
    32768-row bin -- dma_gather indices are int16) in TRANSPOSE mode, which
    lands rows channel-major: G_T[chan, col] in SBUF.
  - PE matmuls reduce each 128-column chunk: out[col, o] = sum_c G_T[c, col]
    * M_T[c, o] -- no DVE work at all.
  - Host scatters per-row results y'[row] back to sample order and does the
    final pair add out[s] = y'[2s] + y'[2s+1] + c (131072 adds on 1 MB --
    negligible host work; the memory-heavy gather and the 67 MFLOP reduction
    stay on device).

Why not indirect_dma_start: on real HW it consumes ONE index per partition
per instruction (the multi-index-per-partition semantics exist only in the
simulator), so bulk gathers would need 128 instructions/core and die on
SWDGE fixed overhead.  dma_gather is the purpose-built bulk gather.
"""

import numpy as np

NUM_NODES = 1_000_000
EMB = 128
B = 65536
NCORES = 8
P = 128
SHARD = NUM_NODES // NCORES       # 125000 rows owned per core
BINROWS = 32768                   # int16 index range per dma_gather
NBINS = 4                         # 4*32768 = 131072 >= 125000 (shard padded)
NROWS_PAD = NBINS * BINROWS       # 131072
BINPAD = 4480                     # per-bin index capacity (35*128; actual max
                                  # occupancy for the seed-0 inputs is 4453;
                                  # host spill-fallback covers any overflow)
NCOLS = NBINS * BINPAD            # 18432 gather columns per core
NCHUNK = BINPAD // P              # 36 matmul chunks per bin

_CACHE: dict = {}


def _build_nc(binpad: int = BINPAD, reps: int = 1):
    """Build the single-core Bass module (same program runs SPMD on 8 cores).

    reps > 1 wraps the gather+matmul+store pipeline in a tc.For_i loop --
    benchmarking only (wall-clock slope over reps isolates per-iteration HW
    time from NEFF-launch/tunnel overhead)."""
    import contextlib
    import concourse.bass as bass
    import concourse.bacc as bacc
    import concourse.mybir as mybir
    import concourse.tile as tile

    nchunk = binpad // P
    # Bacc (not plain Bass): its compile() runs generate_event_semaphores,
    # which splits multi-wait sync_infos -- TRN2 allows at most 1 wait per
    # instruction and walrus codegen hard-errors otherwise.
    nc = bacc.Bacc("TRN2", target_bir_lowering=False, debug=False)
    embs = nc.dram_tensor("embs", [NROWS_PAD, EMB], mybir.dt.float16,
                          kind="ExternalInput")
    idxs_d = nc.dram_tensor("idxs", [P, NBINS * binpad // 16], mybir.dt.int16,
                            kind="ExternalInput")
    mcat_d = nc.dram_tensor("mcat", [P, 2], mybir.dt.float16, kind="ExternalInput")
    y_d = nc.dram_tensor("y", [P, NBINS * nchunk * 2], mybir.dt.float32,
                         kind="ExternalOutput")

    with tile.TileContext(nc) as tc:
        with tc.tile_pool(name="const", bufs=1) as cpool, \
             tc.tile_pool(name="gat", bufs=2) as gpool, \
             tc.tile_pool(name="ps", bufs=2, space="PSUM") as ppool:

            idxs_sb = cpool.tile([P, NBINS * binpad // 16], mybir.dt.int16)
            mcat_sb = cpool.tile([P, 2], mybir.dt.float16)
            y_sb = cpool.tile([P, NBINS * nchunk * 2], mybir.dt.float32)

            nc.sync.dma_start(out=idxs_sb[:], in_=idxs_d.ap())
            nc.sync.dma_start(out=mcat_sb[:], in_=mcat_d.ap())

            loop_cm = tc.For_i(0, reps, 1) if reps > 1 else contextlib.nullcontext()
            with loop_cm:
                for b in range(NBINS):
                    gt = gpool.tile([P, 1, binpad], mybir.dt.float16, tag="g")
                    nc.gpsimd.dma_gather(
                        out_ap=gt[:],
                        in_ap=embs.ap()[b * BINROWS:(b + 1) * BINROWS, :],
                        idxs_ap=idxs_sb[:, b * (binpad // 16):(b + 1) * (binpad // 16)],
                        num_idxs=binpad,
                        num_idxs_reg=binpad,
                        elem_size=EMB,
                        transpose=True,
                        # single_packet packs one gather's descriptors into one
                        # DMA packet; packets cap at 64 descriptors (16 idxs
                        # each), so >896 idxs requires multi-packet mode --
                        # single_packet=True hard-crashes the NEFF at 4608.
                        single_packet=False,
                    )
                    pt = ppool.tile([P, nchunk * 2], mybir.dt.float32, tag="p")
                    for j in range(nchunk):
                        nc.tensor.matmul(
                            out=pt[:, 2 * j:2 * j + 2],
                            lhsT=gt[:, 0, j * P:(j + 1) * P],
                            rhs=mcat_sb[:],
                            start=True, stop=True)
                    nc.vector.tensor_copy(
                        out=y_sb[:, b * nchunk * 2:(b + 1) * nchunk * 2],
                        in_=pt[:])
                nc.sync.dma_start(out=y_d.ap(), in_=y_sb[:])

    nc.compile()
    return nc


def _fold_weights(W1_rel, b1, W1_root, W2_rel, b2, W2_root, fc_W, fc_b):
    M2 = (W2_rel + W2_root).astype(np.float64)
    M1 = (W1_rel + W1_root).astype(np.float64)
    M = 0.5 * fc_W.astype(np.float64) @ M2 @ M1            # [2, 128]
    c = (fc_W.astype(np.float64) @ (M2 @ b1.astype(np.float64)
                                    + b2.astype(np.float64))
         + fc_b.astype(np.float64))                        # [2]
    return M.astype(np.float32), c.astype(np.float32)


def _route(x, binpad: int = BINPAD):
    """Host-side routing: per core, bin-compacted int16 index arrays plus the
    column -> flat-row-slot map for reassembly.  Returns (in-parts, col_pos
    list, spill list of (slot, row) that exceeded a bin's capacity)."""
    xf = np.ascontiguousarray(np.asarray(x).astype(np.int64).reshape(-1))  # [131072]
    core = xf // SHARD
    idxs_list, colpos_list, spills = [], [], []
    for m in range(NCORES):
        sel = np.nonzero(core == m)[0]
        loc = xf[sel] - m * SHARD                       # [nm] in [0, 125000)
        b = loc // BINROWS                              # bin 0..3
        order = np.argsort(b, kind="stable")
        loc_o, pos_o, b_o = loc[order], sel[order], b[order]
        idx16 = np.zeros((NBINS, binpad), np.int16)
        colpos = np.full((NBINS, binpad), -1, np.int64)
        for bb in range(NBINS):
            seg = loc_o[b_o == bb] - bb * BINROWS
            pp = pos_o[b_o == bb]
            n = len(seg)
            if n > binpad:                              # ~impossible; exact fallback
                for s_loc, s_pos in zip(seg[binpad:], pp[binpad:]):
                    spills.append((int(s_pos), int(m * SHARD + bb * BINROWS + s_loc)))
                seg, pp, n = seg[:binpad], pp[:binpad], binpad
            idx16[bb, :n] = seg.astype(np.int16)
            colpos[bb, :n] = pp
        # dma_gather consumes indices wrapped over 16 partitions: flat index i
        # lives at [i % 16, i // 16]; replicate to all 8 groups of 16.
        wrapped = np.concatenate(
            [idx16[bb].reshape(binpad // 16, 16).T for bb in range(NBINS)], axis=1)
        idxs_list.append(np.tile(wrapped, (8, 1)).astype(np.int16))  # [128, NBINS*binpad/16]
        colpos_list.append(colpos)
    return idxs_list, colpos_list, spills


def _per_core_inputs(x, emb16, M, binpad: int = BINPAD):
    mcat = np.ascontiguousarray(M.T.astype(np.float16))    # [128, 2]
    idxs_list, colpos_list, spills = _route(x, binpad)
    in_maps = []
    for m in range(NCORES):
        embs_m = np.zeros((NROWS_PAD, EMB), np.float16)
        embs_m[:SHARD] = emb16[m * SHARD:(m + 1) * SHARD]
        in_maps.append({
            "embs": embs_m,
            "idxs": idxs_list[m],
            "mcat": mcat,
        })
    return in_maps, colpos_list, spills


def _assemble(y_list, colpos_list, spills, emb, M, c, binpad: int = BINPAD,
              n_rows: int = B * 2):
    nchunk = binpad // P
    yv = np.zeros((n_rows, 2), np.float32)
    for m in range(NCORES):
        # y[p, (b*nchunk + j)*2 + o] = y' of column b*binpad + j*128 + p
        y4 = y_list[m].reshape(P, NBINS, nchunk, 2)
        yflat = y4.transpose(1, 2, 0, 3).reshape(NBINS * binpad, 2)
        cp = colpos_list[m].reshape(-1)
        valid = cp >= 0
        yv[cp[valid]] = yflat[valid]
    for slot, row in spills:
        yv[slot] = (M.astype(np.float64) @ emb[row].astype(np.float64)).astype(np.float32)
    out = yv[0::2] + yv[1::2] + c.astype(np.float32)
    return out


def _get_compiled(binpad: int = BINPAD, reps: int = 1):
    key = ("nc", binpad, reps)
    if key not in _CACHE:
        _CACHE[key] = _build_nc(binpad, reps)
    return _CACHE[key]


def run(x, emb, M, c, trace=False, **spmd_kwargs):
    """Run the SPMD kernel on 8 cores; returns (out [65536,2] f32, results)."""
    from concourse.bass_utils import run_bass_kernel_spmd
    nc = _get_compiled()
    emb16 = emb.astype(np.float16)
    in_maps, colpos_list, spills = _per_core_inputs(x, emb16, M)
    res = run_bass_kernel_spmd(nc, in_maps, core_ids=list(range(NCORES)),
                               trace=trace, **spmd_kwargs)
    y_list = [res.results[m]["y"] for m in range(NCORES)]
    out = _assemble(y_list, colpos_list, spills, emb, M, c)
    return out, res


def kernel(x, emb, W1_rel, b1, W1_root, W2_rel, b2, W2_root, fc_W, fc_b):
    x = np.asarray(x)
    emb = np.asarray(emb, dtype=np.float32)
    M, c = _fold_weights(np.asarray(W1_rel), np.asarray(b1), np.asarray(W1_root),
                         np.asarray(W2_rel), np.asarray(b2), np.asarray(W2_root),
                         np.asarray(fc_W), np.asarray(fc_b))
    out, _ = run(x, emb, M, c)
    return out
